# revision 15
# baseline (speedup 1.0000x reference)
"""MoE ConditionalFeedForward (SwiGLU expert FFN) for 8 Trainium2 NeuronCores.

Expert-parallel: core e holds expert e's weights (host pre-packed) and
computes the full SwiGLU FFN for ALL 16 tokens densely:
    y_e = (silu(x @ w1[e].T) * (x @ w3[e].T)) @ w2[e].T        [16, 1024]
The (token, slot) -> expert routing is a pure gather done on the host:
    out[t, a] = y_{expert_indices[t, a]}[t]
Since T=16 <= 128, computing all tokens per expert costs the same PE time as
computing only the routed ones, and weight DMA dominates (memory regime).

All matmuls stream the weight matrix as the moving operand (N=512 free dim)
with the small [128, 16] activation tile stationary, so the PE streams each
weight element exactly once.

Weights are packed on the host into a [128, 22528] layout where each DMA
block is fully contiguous per partition row (16 KB lines) - 8x fewer DMA
descriptors than the naive transposed layout, which otherwise saturates the
sync engine's HWDGE descriptor generation.
"""

import os
import threading

import numpy as np

NUM_EXPERTS = 8
INTER = 2816
DIM = 1024
T = 16
A = 2
P = 128
N_CORES = 8
KD = DIM // P  # 8 contraction chunks for stage 1
KI = INTER // P  # 22 contraction chunks for stage 2
J_TILES = [(0, 512), (512, 512), (1024, 512), (1536, 512), (2048, 512), (2560, 256)]
G2_SIZES = [6, 6, 6, 4]  # stage-2 group sizes in 128-chunks (sum = KI)
WCOLS = KD * INTER  # 22528 packed columns per partition

# "f32" (float32r end-to-end), "fp16", or "bf16"
WEIGHT_MODE = os.environ.get("KERNEL_WEIGHT_MODE", "fp16")

_lock = threading.Lock()
_nc_cache = {}
LAST_RESULTS = None  # BassKernelResults of the most recent kernel() call


def _np_wdt(mode):
    if mode == "f32":
        return np.float32
    if mode == "fp16":
        return np.float16
    import ml_dtypes

    return ml_dtypes.bfloat16


def _pack_w13(w1, w3, np_wdt):
    """[INTER, DIM] x2 -> [P, 2*WCOLS]; block j is [P, 2, KD, jsz] per row.

    packed[p, 2*KD*joff + (m*KD + k)*jsz + c] = w_m[joff + c, k*P + p]
    """
    blocks = []
    for joff, jsz in J_TILES:
        b1 = w1[joff : joff + jsz].reshape(jsz, KD, P).transpose(2, 1, 0)
        b3 = w3[joff : joff + jsz].reshape(jsz, KD, P).transpose(2, 1, 0)
        blk = np.stack([b1, b3], axis=1)  # [P, 2, KD, jsz]
        blocks.append(np.ascontiguousarray(blk, dtype=np_wdt).reshape(P, 2 * KD * jsz))
    return np.ascontiguousarray(np.concatenate(blocks, axis=1))


def _pack_w2(w2, np_wdt):
    """[DIM, INTER] -> [P, WCOLS]; group g is [P, gc, DIM] contiguous per row.

    packed[p, (ic0 + k)*DIM + c] = w2[c, (ic0 + k)*P + p]
    """
    w2t = w2.T  # [INTER, DIM]
    groups = []
    ic0 = 0
    for gc in G2_SIZES:
        grp = (
            w2t[ic0 * P : (ic0 + gc) * P].reshape(gc, P, DIM).transpose(1, 0, 2)
        )
        groups.append(np.ascontiguousarray(grp, dtype=np_wdt).reshape(P, gc * DIM))
        ic0 += gc
    return np.ascontiguousarray(np.concatenate(groups, axis=1))


def _build_nc(mode: str):
    import concourse.bass as bass
    import concourse.tile as tile
    from concourse import bacc, mybir

    f32 = mybir.dt.float32
    if mode == "f32":
        # float32r: same 4-byte layout as f32, single-pass PE matmul. The BIR
        # verifier requires matmul operands *declared* f32r end-to-end.
        wdt = mybir.dt.float32r
    elif mode == "fp16":
        wdt = mybir.dt.float16
    else:
        wdt = mybir.dt.bfloat16

    # Bacc (not plain Bass): its compile() runs move_matmul_waits_to_ldweights
    # + generate_event_semaphores, required because fp32r matmuls lower to a
    # self-loading LDWEIGHTS struct with a single sync-wait slot.
    nc = bacc.Bacc()
    xt_d = nc.declare_dram_parameter("xt", [P, KD * T], wdt, isOutput=False)
    w13_d = nc.declare_dram_parameter("w13p", [P, 2 * WCOLS], wdt, isOutput=False)
    w2_d = nc.declare_dram_parameter("w2p", [P, WCOLS], wdt, isOutput=False)
    eye_d = nc.declare_dram_parameter("eye", [T, T], f32, isOutput=False)
    out_d = nc.declare_dram_parameter("out", [T, DIM], f32, isOutput=True)

    with tile.TileContext(nc) as tc:
        with (
            tc.tile_pool(name="const", bufs=1) as cpool,
            tc.tile_pool(name="wpool", bufs=3) as wp,
            tc.tile_pool(name="hp", bufs=1) as hp,
            tc.tile_pool(name="ep", bufs=3) as ep,
            tc.tile_pool(name="outp", bufs=1) as outp,
            tc.tile_pool(name="ps1", bufs=4, space="PSUM") as ps1p,
            tc.tile_pool(name="pst", bufs=2, space="PSUM") as pstp,
            tc.tile_pool(name="pso", bufs=1, space="PSUM") as psop,
        ):
            # small loads go on the scalar (ACT) HWDGE ring so the sync ring
            # streams nothing but weight blocks
            xt_sb = cpool.tile([P, KD, T], wdt)
            nc.scalar.dma_start(xt_sb[:], xt_d.rearrange("p (k t) -> p k t", k=KD))
            eye_sb = cpool.tile([T, T], f32)
            nc.scalar.dma_start(eye_sb[:], eye_d[:])

            h_sb = hp.tile([T, INTER], f32)
            hT_sb = hp.tile([P, KI * T], wdt)

            # ---- stage 1: x1 = x@w1.T, x3 = x@w3.T, h = silu(x1)*x3 ----
            for joff, jsz in J_TILES:
                col0 = 2 * KD * joff
                wb = wp.tile([P, 2, KD, jsz], wdt, tag="w13b")
                nc.sync.dma_start(
                    wb[:],
                    w13_d[:, col0 : col0 + 2 * KD * jsz].rearrange(
                        "p (m k c) -> p m k c", m=2, k=KD
                    ),
                )

                ps1 = ps1p.tile([T, jsz], f32, tag="psa")
                ps3 = ps1p.tile([T, jsz], f32, tag="psa")
                for k in range(KD):
                    lhs = xt_sb[:, k, :]
                    nc.tensor.matmul(
                        ps1[:], lhs, wb[:, 0, k, :],
                        start=(k == 0), stop=(k == KD - 1),
                    )
                    nc.tensor.matmul(
                        ps3[:], lhs, wb[:, 1, k, :],
                        start=(k == 0), stop=(k == KD - 1),
                    )

                # silu(x1)*x3 = x1*sigmoid(x1)*x3 (no Silu LUT on trn2 ACT)
                sg = ep.tile([T, jsz], f32, tag="sg")
                nc.scalar.activation(
                    sg[:], ps1[:], mybir.ActivationFunctionType.Sigmoid
                )
                tmp = ep.tile([T, jsz], f32, tag="tmp")
                nc.vector.tensor_mul(tmp[:], sg[:], ps3[:])
                nc.vector.tensor_mul(h_sb[:, joff : joff + jsz], tmp[:], ps1[:])

                # transpose the freshly finished h columns into [INTER, T]
                for c in range(joff // P, (joff + jsz) // P):
                    pt = pstp.tile([P, T], f32, tag="pt")
                    nc.tensor.transpose(
                        pt[:], h_sb[:, c * P : (c + 1) * P], eye_sb[:]
                    )
                    nc.vector.tensor_copy(hT_sb[:, c * T : (c + 1) * T], pt[:])

            # ---- stage 2: out = h @ w2.T ----
            pso0 = psop.tile([T, 512], f32)
            pso1 = psop.tile([T, 512], f32)
            psos = [pso0, pso1]
            ic0 = 0
            for gc in G2_SIZES:
                col0 = ic0 * DIM
                w2b = wp.tile([P, gc, DIM], wdt, tag="w2b")
                nc.sync.dma_start(
                    w2b[:],
                    w2_d[:, col0 : col0 + gc * DIM].rearrange("p (k c) -> p k c", k=gc),
                )
                for k in range(gc):
                    ic = ic0 + k
                    lhs = hT_sb[:, ic * T : (ic + 1) * T]
                    for dd in range(2):
                        nc.tensor.matmul(
                            psos[dd][:], lhs, w2b[:, k, dd * 512 : (dd + 1) * 512],
                            start=(ic == 0), stop=(ic == KI - 1),
                        )
                ic0 += gc

            out_sb = outp.tile([T, DIM], f32)
            nc.vector.tensor_copy(out_sb[:, 0:512], pso0[:])
            nc.vector.tensor_copy(out_sb[:, 512:1024], pso1[:])
            nc.scalar.dma_start(out_d[:], out_sb[:])

    nc.compile()
    return nc


def _get_nc(mode: str):
    with _lock:
        if mode not in _nc_cache:
            _nc_cache[mode] = _build_nc(mode)
        return _nc_cache[mode]


def _harden_trace_path():
    """Make run_bass_kernel_spmd's trace path survive this container.

    The agent image's antenv lacks axon_hooks (bass_utils crashes importing it
    when BASS_TRACE is set), and the artifact upload needs fish-bucket creds.
    Inject the same ctypes NTFF hook the full boot would have registered, and
    fall back to a local path if the artifact upload fails.
    """
    import sys
    import types

    try:
        import antenv  # noqa: F401
        import antenv.axon_hooks  # noqa: F401
    except ImportError:
        mod = types.ModuleType("antenv.axon_hooks")
        state = {"hook": None}
        try:
            import contextlib
            import ctypes

            lib = ctypes.CDLL("/opt/axon/libaxon_pjrt.so")
            if hasattr(lib, "axon_start_nrt_profile"):
                lib.axon_start_nrt_profile.argtypes = [
                    ctypes.POINTER(ctypes.c_int64),
                    ctypes.c_size_t,
                ]
                lib.axon_start_nrt_profile.restype = ctypes.c_int64
                lib.axon_stop_nrt_profile.argtypes = [ctypes.c_char_p]
                lib.axon_stop_nrt_profile.restype = ctypes.c_int64

                @contextlib.contextmanager
                def _hook(output_dir, device_ids):
                    import jax

                    jax.devices()
                    if device_ids:
                        ids = (ctypes.c_int64 * len(device_ids))(*device_ids)
                        rc = lib.axon_start_nrt_profile(ids, len(device_ids))
                    else:
                        rc = lib.axon_start_nrt_profile(None, 0)
                    if rc != 0:
                        raise RuntimeError(f"axon_start_nrt_profile rc={rc}")
                    try:
                        yield
                    finally:
                        lib.axon_stop_nrt_profile(str(output_dir).encode())

                state["hook"] = _hook
        except OSError:
            pass
        mod.get_axon_ntff_profile_hook = lambda: state["hook"]
        mod.set_axon_ntff_profile_hook = lambda h: state.update(hook=h)
        sys.modules["antenv.axon_hooks"] = mod
        try:
            import antenv

            antenv.axon_hooks = mod
        except ImportError:
            pass

    import concourse.bass_utils as bu

    if not getattr(bu.upload_artifacts, "_safe_wrapped", False):
        orig = bu.upload_artifacts

        def _safe_upload(tmpdir):
            try:
                return orig(tmpdir)
            except Exception:
                return f"local://{tmpdir}"

        _safe_upload._safe_wrapped = True
        bu.upload_artifacts = _safe_upload


def kernel(**inputs: np.ndarray) -> np.ndarray:
    global LAST_RESULTS
    _harden_trace_path()
    from concourse.bass_utils import run_bass_kernel_spmd

    mode = WEIGHT_MODE
    x = np.asarray(inputs["x"], dtype=np.float32)
    expert_indices = np.asarray(inputs["expert_indices"]).astype(np.int64)
    w1 = np.asarray(inputs["w1"], dtype=np.float32)
    w2 = np.asarray(inputs["w2"], dtype=np.float32)
    w3 = np.asarray(inputs["w3"], dtype=np.float32)

    np_wdt = _np_wdt(mode)

    # x.T laid out [128 partitions, 8 k-chunks * 16 tokens]
    xt = np.ascontiguousarray(
        x.T.reshape(KD, P, T).transpose(1, 0, 2), dtype=np_wdt
    ).reshape(P, KD * T)
    eye = np.eye(T, dtype=np.float32)

    in_maps = []
    for e in range(N_CORES):
        in_maps.append(
            {
                "xt": xt,
                "w13p": _pack_w13(w1[e], w3[e], np_wdt),
                "w2p": _pack_w2(w2[e], np_wdt),
                "eye": eye,
            }
        )

    nc = _get_nc(mode)
    res = run_bass_kernel_spmd(nc, in_maps, core_ids=list(range(N_CORES)))
    LAST_RESULTS = res

    y = np.stack([res.results[e]["out"] for e in range(N_CORES)])  # [8, 16, 1024]
    out = y[expert_indices, np.arange(T)[:, None]]  # [16, 2, 1024]
    return np.ascontiguousarray(out, dtype=np.float32)


# revision 18
# speedup vs baseline: 1.0643x; 1.0643x over previous
"""MoE ConditionalFeedForward (SwiGLU expert FFN) for 8 Trainium2 NeuronCores.

Expert-parallel: core e holds expert e's weights (host pre-packed) and
computes the full SwiGLU FFN for ALL 16 tokens densely:
    y_e = (silu(x @ w1[e].T) * (x @ w3[e].T)) @ w2[e].T        [16, 1024]
The (token, slot) -> expert routing is a pure gather done on the host:
    out[t, a] = y_{expert_indices[t, a]}[t]
Since T=16 <= 128, computing all tokens per expert costs the same PE time as
computing only the routed ones, and weight DMA dominates (memory regime).

All matmuls stream the weight matrix as the moving operand (N=512 free dim)
with the small [128, 16] activation tile stationary, so the PE streams each
weight element exactly once.

Weights are packed on the host into a [128, 22528] layout where each DMA
block is fully contiguous per partition row (16 KB lines) - 8x fewer DMA
descriptors than the naive transposed layout, which otherwise saturates the
sync engine's HWDGE descriptor generation.
"""

import os
import threading

import numpy as np

NUM_EXPERTS = 8
INTER = 2816
DIM = 1024
T = 16
A = 2
P = 128
N_CORES = 8
KD = DIM // P  # 8 contraction chunks for stage 1
KI = INTER // P  # 22 contraction chunks for stage 2
J_TILES = [(0, 512), (512, 512), (1024, 512), (1536, 512), (2048, 512), (2560, 256)]
G2_SIZES = [6, 6, 6, 4]  # stage-2 group sizes in 128-chunks (sum = KI)
WCOLS = KD * INTER  # 22528 packed columns per partition

# "f32" (float32r end-to-end), "fp16", or "bf16"
WEIGHT_MODE = os.environ.get("KERNEL_WEIGHT_MODE", "fp16")

_lock = threading.Lock()
_nc_cache = {}
LAST_RESULTS = None  # BassKernelResults of the most recent kernel() call


def _np_wdt(mode):
    if mode == "f32":
        return np.float32
    if mode == "fp16":
        return np.float16
    import ml_dtypes

    return ml_dtypes.bfloat16


def _pack_w13(w1, w3, np_wdt):
    """[INTER, DIM] x2 -> [P, 2*WCOLS]; block j is [P, 2, KD, jsz] per row.

    packed[p, 2*KD*joff + (m*KD + k)*jsz + c] = w_m[joff + c, k*P + p]
    """
    blocks = []
    for joff, jsz in J_TILES:
        b1 = w1[joff : joff + jsz].reshape(jsz, KD, P).transpose(2, 1, 0)
        b3 = w3[joff : joff + jsz].reshape(jsz, KD, P).transpose(2, 1, 0)
        blk = np.stack([b1, b3], axis=1)  # [P, 2, KD, jsz]
        blocks.append(np.ascontiguousarray(blk, dtype=np_wdt).reshape(P, 2 * KD * jsz))
    return np.ascontiguousarray(np.concatenate(blocks, axis=1))


def _pack_w2(w2, np_wdt):
    """[DIM, INTER] -> [P, WCOLS]; group g is [P, gc, DIM] contiguous per row.

    packed[p, (ic0 + k)*DIM + c] = w2[c, (ic0 + k)*P + p]
    """
    w2t = w2.T  # [INTER, DIM]
    groups = []
    ic0 = 0
    for gc in G2_SIZES:
        grp = (
            w2t[ic0 * P : (ic0 + gc) * P].reshape(gc, P, DIM).transpose(1, 0, 2)
        )
        groups.append(np.ascontiguousarray(grp, dtype=np_wdt).reshape(P, gc * DIM))
        ic0 += gc
    return np.ascontiguousarray(np.concatenate(groups, axis=1))


def _build_nc(mode: str):
    import concourse.bass as bass
    import concourse.tile as tile
    from concourse import bacc, mybir

    f32 = mybir.dt.float32
    if mode == "f32":
        # float32r: same 4-byte layout as f32, single-pass PE matmul. The BIR
        # verifier requires matmul operands *declared* f32r end-to-end.
        wdt = mybir.dt.float32r
    elif mode == "fp16":
        wdt = mybir.dt.float16
    else:
        wdt = mybir.dt.bfloat16

    # Bacc (not plain Bass): its compile() runs move_matmul_waits_to_ldweights
    # + generate_event_semaphores, required because fp32r matmuls lower to a
    # self-loading LDWEIGHTS struct with a single sync-wait slot.
    nc = bacc.Bacc()
    xt_d = nc.declare_dram_parameter("xt", [P, KD * T], wdt, isOutput=False)
    w13_d = nc.declare_dram_parameter("w13p", [P, 2 * WCOLS], wdt, isOutput=False)
    w2_d = nc.declare_dram_parameter("w2p", [P, WCOLS], wdt, isOutput=False)
    eye_d = nc.declare_dram_parameter("eye", [T, T], f32, isOutput=False)
    out_d = nc.declare_dram_parameter("out", [T, DIM], f32, isOutput=True)

    with tile.TileContext(nc) as tc:
        with (
            tc.tile_pool(name="const", bufs=1) as cpool,
            tc.tile_pool(name="wpool", bufs=1) as wp,
            tc.tile_pool(name="hp", bufs=1) as hp,
            tc.tile_pool(name="ep", bufs=3) as ep,
            tc.tile_pool(name="outp", bufs=1) as outp,
            tc.tile_pool(name="ps1", bufs=4, space="PSUM") as ps1p,
            tc.tile_pool(name="pst", bufs=2, space="PSUM") as pstp,
            tc.tile_pool(name="pso", bufs=1, space="PSUM") as psop,
        ):
            # small loads go on the scalar (ACT) HWDGE ring so the sync ring
            # streams nothing but weight blocks
            xt_sb = cpool.tile([P, KD, T], wdt)
            nc.scalar.dma_start(xt_sb[:], xt_d.rearrange("p (k t) -> p k t", k=KD))
            eye_sb = cpool.tile([T, T], f32)
            nc.scalar.dma_start(eye_sb[:], eye_d[:])

            h_sb = hp.tile([T, INTER], f32)
            hT_sb = hp.tile([P, KI * T], wdt)

            # ---- stage 1: x1 = x@w1.T, x3 = x@w3.T, h = silu(x1)*x3 ----
            for joff, jsz in J_TILES:
                col0 = 2 * KD * joff
                # all 6 blocks resident (16 KB/partition each at fp16): the
                # sync ring issues every weight DMA up front, no slot waits
                wb = wp.tile([P, 2, KD, jsz], wdt, tag="w13b", bufs=len(J_TILES))
                nc.sync.dma_start(
                    wb[:],
                    w13_d[:, col0 : col0 + 2 * KD * jsz].rearrange(
                        "p (m k c) -> p m k c", m=2, k=KD
                    ),
                )

                ps1 = ps1p.tile([T, jsz], f32, tag="psa")
                ps3 = ps1p.tile([T, jsz], f32, tag="psa")
                for k in range(KD):
                    lhs = xt_sb[:, k, :]
                    nc.tensor.matmul(
                        ps1[:], lhs, wb[:, 0, k, :],
                        start=(k == 0), stop=(k == KD - 1),
                    )
                    nc.tensor.matmul(
                        ps3[:], lhs, wb[:, 1, k, :],
                        start=(k == 0), stop=(k == KD - 1),
                    )

                # silu(x1)*x3 = x1*sigmoid(x1)*x3 (no Silu LUT on trn2 ACT)
                sg = ep.tile([T, jsz], f32, tag="sg")
                nc.scalar.activation(
                    sg[:], ps1[:], mybir.ActivationFunctionType.Sigmoid
                )
                tmp = ep.tile([T, jsz], f32, tag="tmp")
                nc.vector.tensor_mul(tmp[:], sg[:], ps3[:])
                nc.vector.tensor_mul(h_sb[:, joff : joff + jsz], tmp[:], ps1[:])

                # transpose the freshly finished h columns into [INTER, T]
                for c in range(joff // P, (joff + jsz) // P):
                    pt = pstp.tile([P, T], f32, tag="pt")
                    nc.tensor.transpose(
                        pt[:], h_sb[:, c * P : (c + 1) * P], eye_sb[:]
                    )
                    nc.vector.tensor_copy(hT_sb[:, c * T : (c + 1) * T], pt[:])

            # ---- stage 2: out = h @ w2.T ----
            pso0 = psop.tile([T, 512], f32)
            pso1 = psop.tile([T, 512], f32)
            psos = [pso0, pso1]
            ic0 = 0
            for gc in G2_SIZES:
                col0 = ic0 * DIM
                w2b = wp.tile([P, gc, DIM], wdt, tag="w2b", bufs=len(G2_SIZES))
                nc.sync.dma_start(
                    w2b[:],
                    w2_d[:, col0 : col0 + gc * DIM].rearrange("p (k c) -> p k c", k=gc),
                )
                for k in range(gc):
                    ic = ic0 + k
                    lhs = hT_sb[:, ic * T : (ic + 1) * T]
                    for dd in range(2):
                        nc.tensor.matmul(
                            psos[dd][:], lhs, w2b[:, k, dd * 512 : (dd + 1) * 512],
                            start=(ic == 0), stop=(ic == KI - 1),
                        )
                ic0 += gc

            out_sb = outp.tile([T, DIM], f32)
            nc.vector.tensor_copy(out_sb[:, 0:512], pso0[:])
            nc.vector.tensor_copy(out_sb[:, 512:1024], pso1[:])
            nc.scalar.dma_start(out_d[:], out_sb[:])

    nc.compile()
    return nc


def _get_nc(mode: str):
    with _lock:
        if mode not in _nc_cache:
            _nc_cache[mode] = _build_nc(mode)
        return _nc_cache[mode]


def _harden_trace_path():
    """Make run_bass_kernel_spmd's trace path survive this container.

    The agent image's antenv lacks axon_hooks (bass_utils crashes importing it
    when BASS_TRACE is set), and the artifact upload needs fish-bucket creds.
    Inject the same ctypes NTFF hook the full boot would have registered, and
    fall back to a local path if the artifact upload fails.
    """
    import sys
    import types

    try:
        import antenv  # noqa: F401
        import antenv.axon_hooks  # noqa: F401
    except ImportError:
        mod = types.ModuleType("antenv.axon_hooks")
        state = {"hook": None}
        try:
            import contextlib
            import ctypes

            lib = ctypes.CDLL("/opt/axon/libaxon_pjrt.so")
            if hasattr(lib, "axon_start_nrt_profile"):
                lib.axon_start_nrt_profile.argtypes = [
                    ctypes.POINTER(ctypes.c_int64),
                    ctypes.c_size_t,
                ]
                lib.axon_start_nrt_profile.restype = ctypes.c_int64
                lib.axon_stop_nrt_profile.argtypes = [ctypes.c_char_p]
                lib.axon_stop_nrt_profile.restype = ctypes.c_int64

                @contextlib.contextmanager
                def _hook(output_dir, device_ids):
                    import jax

                    jax.devices()
                    if device_ids:
                        ids = (ctypes.c_int64 * len(device_ids))(*device_ids)
                        rc = lib.axon_start_nrt_profile(ids, len(device_ids))
                    else:
                        rc = lib.axon_start_nrt_profile(None, 0)
                    if rc != 0:
                        raise RuntimeError(f"axon_start_nrt_profile rc={rc}")
                    try:
                        yield
                    finally:
                        lib.axon_stop_nrt_profile(str(output_dir).encode())

                state["hook"] = _hook
        except OSError:
            pass
        mod.get_axon_ntff_profile_hook = lambda: state["hook"]
        mod.set_axon_ntff_profile_hook = lambda h: state.update(hook=h)
        sys.modules["antenv.axon_hooks"] = mod
        try:
            import antenv

            antenv.axon_hooks = mod
        except ImportError:
            pass

    import concourse.bass_utils as bu

    if not getattr(bu.upload_artifacts, "_safe_wrapped", False):
        orig = bu.upload_artifacts

        def _safe_upload(tmpdir):
            try:
                return orig(tmpdir)
            except Exception:
                return f"local://{tmpdir}"

        _safe_upload._safe_wrapped = True
        bu.upload_artifacts = _safe_upload


def kernel(**inputs: np.ndarray) -> np.ndarray:
    global LAST_RESULTS
    _harden_trace_path()
    from concourse.bass_utils import run_bass_kernel_spmd

    mode = WEIGHT_MODE
    x = np.asarray(inputs["x"], dtype=np.float32)
    expert_indices = np.asarray(inputs["expert_indices"]).astype(np.int64)
    w1 = np.asarray(inputs["w1"], dtype=np.float32)
    w2 = np.asarray(inputs["w2"], dtype=np.float32)
    w3 = np.asarray(inputs["w3"], dtype=np.float32)

    np_wdt = _np_wdt(mode)

    # x.T laid out [128 partitions, 8 k-chunks * 16 tokens]
    xt = np.ascontiguousarray(
        x.T.reshape(KD, P, T).transpose(1, 0, 2), dtype=np_wdt
    ).reshape(P, KD * T)
    eye = np.eye(T, dtype=np.float32)

    in_maps = []
    for e in range(N_CORES):
        in_maps.append(
            {
                "xt": xt,
                "w13p": _pack_w13(w1[e], w3[e], np_wdt),
                "w2p": _pack_w2(w2[e], np_wdt),
                "eye": eye,
            }
        )

    nc = _get_nc(mode)
    res = run_bass_kernel_spmd(nc, in_maps, core_ids=list(range(N_CORES)))
    LAST_RESULTS = res

    y = np.stack([res.results[e]["out"] for e in range(N_CORES)])  # [8, 16, 1024]
    out = y[expert_indices, np.arange(T)[:, None]]  # [16, 2, 1024]
    return np.ascontiguousarray(out, dtype=np.float32)


# revision 19
# speedup vs baseline: 1.1638x; 1.0935x over previous
"""MoE ConditionalFeedForward (SwiGLU expert FFN) for 8 Trainium2 NeuronCores.

Expert-parallel: core e holds expert e's weights (host pre-packed) and
computes the full SwiGLU FFN for ALL 16 tokens densely:
    y_e = (silu(x @ w1[e].T) * (x @ w3[e].T)) @ w2[e].T        [16, 1024]
The (token, slot) -> expert routing is a pure gather done on the host:
    out[t, a] = y_{expert_indices[t, a]}[t]
Since T=16 <= 128, computing all tokens per expert costs the same PE time as
computing only the routed ones, and weight DMA dominates (memory regime).

All matmuls stream the weight matrix as the moving operand (N=512 free dim)
with the small [128, 16] activation tile stationary, so the PE streams each
weight element exactly once.

Weights are packed on the host into a [128, 22528] layout where each DMA
block is fully contiguous per partition row (16 KB lines) - 8x fewer DMA
descriptors than the naive transposed layout, which otherwise saturates the
sync engine's HWDGE descriptor generation.
"""

import os
import threading

import numpy as np

NUM_EXPERTS = 8
INTER = 2816
DIM = 1024
T = 16
A = 2
P = 128
N_CORES = 8
KD = DIM // P  # 8 contraction chunks for stage 1
KI = INTER // P  # 22 contraction chunks for stage 2
J_TILES = [(0, 512), (512, 512), (1024, 512), (1536, 512), (2048, 512), (2560, 256)]
G2_SIZES = [6, 6, 6, 4]  # stage-2 group sizes in 128-chunks (sum = KI)
WCOLS = KD * INTER  # 22528 packed columns per partition

# "f32" (float32r end-to-end), "fp16", or "bf16"
WEIGHT_MODE = os.environ.get("KERNEL_WEIGHT_MODE", "fp16")

_lock = threading.Lock()
_nc_cache = {}
LAST_RESULTS = None  # BassKernelResults of the most recent kernel() call


def _np_wdt(mode):
    if mode == "f32":
        return np.float32
    if mode == "fp16":
        return np.float16
    import ml_dtypes

    return ml_dtypes.bfloat16


def _pack_w13(w1, w3, np_wdt):
    """[INTER, DIM] x2 -> [P, 2*WCOLS]; block j is [P, 2, KD, jsz] per row.

    packed[p, 2*KD*joff + (m*KD + k)*jsz + c] = w_m[joff + c, k*P + p]
    """
    blocks = []
    for joff, jsz in J_TILES:
        b1 = w1[joff : joff + jsz].reshape(jsz, KD, P).transpose(2, 1, 0)
        b3 = w3[joff : joff + jsz].reshape(jsz, KD, P).transpose(2, 1, 0)
        blk = np.stack([b1, b3], axis=1)  # [P, 2, KD, jsz]
        blocks.append(np.ascontiguousarray(blk, dtype=np_wdt).reshape(P, 2 * KD * jsz))
    return np.ascontiguousarray(np.concatenate(blocks, axis=1))


def _pack_w2(w2, np_wdt):
    """[DIM, INTER] -> [P, WCOLS]; group g is [P, gc, DIM] contiguous per row.

    packed[p, (ic0 + k)*DIM + c] = w2[c, (ic0 + k)*P + p]
    """
    w2t = w2.T  # [INTER, DIM]
    groups = []
    ic0 = 0
    for gc in G2_SIZES:
        grp = (
            w2t[ic0 * P : (ic0 + gc) * P].reshape(gc, P, DIM).transpose(1, 0, 2)
        )
        groups.append(np.ascontiguousarray(grp, dtype=np_wdt).reshape(P, gc * DIM))
        ic0 += gc
    return np.ascontiguousarray(np.concatenate(groups, axis=1))


def _build_nc(mode: str):
    import concourse.bass as bass
    import concourse.tile as tile
    from concourse import bacc, mybir

    f32 = mybir.dt.float32
    if mode == "f32":
        # float32r: same 4-byte layout as f32, single-pass PE matmul. The BIR
        # verifier requires matmul operands *declared* f32r end-to-end.
        wdt = mybir.dt.float32r
    elif mode == "fp16":
        wdt = mybir.dt.float16
    else:
        wdt = mybir.dt.bfloat16

    # Bacc (not plain Bass): its compile() runs move_matmul_waits_to_ldweights
    # + generate_event_semaphores, required because fp32r matmuls lower to a
    # self-loading LDWEIGHTS struct with a single sync-wait slot.
    nc = bacc.Bacc()
    xt_d = nc.declare_dram_parameter("xt", [P, KD * T], wdt, isOutput=False)
    w13_d = nc.declare_dram_parameter("w13p", [P, 2 * WCOLS], wdt, isOutput=False)
    w2_d = nc.declare_dram_parameter("w2p", [P, WCOLS], wdt, isOutput=False)
    eye_d = nc.declare_dram_parameter("eye", [T, T], f32, isOutput=False)
    out_d = nc.declare_dram_parameter("out", [T, DIM], f32, isOutput=True)

    with tile.TileContext(nc) as tc:
        with (
            tc.tile_pool(name="const", bufs=1) as cpool,
            tc.tile_pool(name="wpool", bufs=1) as wp,
            tc.tile_pool(name="hp", bufs=1) as hp,
            tc.tile_pool(name="ep", bufs=3) as ep,
            tc.tile_pool(name="outp", bufs=1) as outp,
            tc.tile_pool(name="ps1", bufs=4, space="PSUM") as ps1p,
            tc.tile_pool(name="pst", bufs=2, space="PSUM") as pstp,
            tc.tile_pool(name="pso", bufs=1, space="PSUM") as psop,
        ):
            # small loads go on the scalar (ACT) HWDGE ring so the sync ring
            # streams nothing but weight blocks
            xt_sb = cpool.tile([P, KD, T], wdt)
            nc.scalar.dma_start(xt_sb[:], xt_d.rearrange("p (k t) -> p k t", k=KD))
            eye_sb = cpool.tile([T, T], f32)
            nc.scalar.dma_start(eye_sb[:], eye_d[:])

            h_sb = hp.tile([T, INTER], f32)
            hT_sb = hp.tile([P, KI * T], wdt)

            # ---- stage 1: x1 = x@w1.T, x3 = x@w3.T, h = silu(x1)*x3 ----
            for jidx, (joff, jsz) in enumerate(J_TILES):
                col0 = 2 * KD * joff
                # all 6 blocks resident (16 KB/partition each at fp16): the
                # sync ring issues every weight DMA up front, no slot waits.
                # Each block arrives as separate w1/w3 half-DMAs (and the
                # first block in quarters) so the PE can start on the first
                # k-chunks while the rest is still in flight.
                wb = wp.tile([P, 2, KD, jsz], wdt, tag="w13b", bufs=len(J_TILES))
                src = w13_d[:, col0 : col0 + 2 * KD * jsz].rearrange(
                    "p (m k c) -> p m k c", m=2, k=KD
                )
                nsplit = 2 if jidx == 0 else 1
                for m in range(2):
                    for sk in range(nsplit):
                        kh = KD // nsplit
                        nc.sync.dma_start(
                            wb[:, m, sk * kh : (sk + 1) * kh, :],
                            src[:, m, sk * kh : (sk + 1) * kh, :],
                        )

                ps1 = ps1p.tile([T, jsz], f32, tag="psa")
                ps3 = ps1p.tile([T, jsz], f32, tag="psa")
                for k in range(KD):
                    lhs = xt_sb[:, k, :]
                    nc.tensor.matmul(
                        ps1[:], lhs, wb[:, 0, k, :],
                        start=(k == 0), stop=(k == KD - 1),
                    )
                    nc.tensor.matmul(
                        ps3[:], lhs, wb[:, 1, k, :],
                        start=(k == 0), stop=(k == KD - 1),
                    )

                # silu(x1)*x3 = x1*sigmoid(x1)*x3 (no Silu LUT on trn2 ACT)
                sg = ep.tile([T, jsz], f32, tag="sg")
                nc.scalar.activation(
                    sg[:], ps1[:], mybir.ActivationFunctionType.Sigmoid
                )
                tmp = ep.tile([T, jsz], f32, tag="tmp")
                nc.vector.tensor_mul(tmp[:], sg[:], ps3[:])
                nc.vector.tensor_mul(h_sb[:, joff : joff + jsz], tmp[:], ps1[:])

                # transpose the freshly finished h columns into [INTER, T]
                for c in range(joff // P, (joff + jsz) // P):
                    pt = pstp.tile([P, T], f32, tag="pt")
                    nc.tensor.transpose(
                        pt[:], h_sb[:, c * P : (c + 1) * P], eye_sb[:]
                    )
                    nc.vector.tensor_copy(hT_sb[:, c * T : (c + 1) * T], pt[:])

            # ---- stage 2: out = h @ w2.T ----
            pso0 = psop.tile([T, 512], f32)
            pso1 = psop.tile([T, 512], f32)
            psos = [pso0, pso1]
            ic0 = 0
            for gc in G2_SIZES:
                col0 = ic0 * DIM
                w2b = wp.tile([P, gc, DIM], wdt, tag="w2b", bufs=len(G2_SIZES))
                nc.sync.dma_start(
                    w2b[:],
                    w2_d[:, col0 : col0 + gc * DIM].rearrange("p (k c) -> p k c", k=gc),
                )
                for k in range(gc):
                    ic = ic0 + k
                    lhs = hT_sb[:, ic * T : (ic + 1) * T]
                    for dd in range(2):
                        nc.tensor.matmul(
                            psos[dd][:], lhs, w2b[:, k, dd * 512 : (dd + 1) * 512],
                            start=(ic == 0), stop=(ic == KI - 1),
                        )
                ic0 += gc

            out_sb = outp.tile([T, DIM], f32)
            nc.vector.tensor_copy(out_sb[:, 0:512], pso0[:])
            nc.vector.tensor_copy(out_sb[:, 512:1024], pso1[:])
            nc.scalar.dma_start(out_d[:], out_sb[:])

    nc.compile()
    return nc


def _get_nc(mode: str):
    with _lock:
        if mode not in _nc_cache:
            _nc_cache[mode] = _build_nc(mode)
        return _nc_cache[mode]


def _harden_trace_path():
    """Make run_bass_kernel_spmd's trace path survive this container.

    The agent image's antenv lacks axon_hooks (bass_utils crashes importing it
    when BASS_TRACE is set), and the artifact upload needs fish-bucket creds.
    Inject the same ctypes NTFF hook the full boot would have registered, and
    fall back to a local path if the artifact upload fails.
    """
    import sys
    import types

    try:
        import antenv  # noqa: F401
        import antenv.axon_hooks  # noqa: F401
    except ImportError:
        mod = types.ModuleType("antenv.axon_hooks")
        state = {"hook": None}
        try:
            import contextlib
            import ctypes

            lib = ctypes.CDLL("/opt/axon/libaxon_pjrt.so")
            if hasattr(lib, "axon_start_nrt_profile"):
                lib.axon_start_nrt_profile.argtypes = [
                    ctypes.POINTER(ctypes.c_int64),
                    ctypes.c_size_t,
                ]
                lib.axon_start_nrt_profile.restype = ctypes.c_int64
                lib.axon_stop_nrt_profile.argtypes = [ctypes.c_char_p]
                lib.axon_stop_nrt_profile.restype = ctypes.c_int64

                @contextlib.contextmanager
                def _hook(output_dir, device_ids):
                    import jax

                    jax.devices()
                    if device_ids:
                        ids = (ctypes.c_int64 * len(device_ids))(*device_ids)
                        rc = lib.axon_start_nrt_profile(ids, len(device_ids))
                    else:
                        rc = lib.axon_start_nrt_profile(None, 0)
                    if rc != 0:
                        raise RuntimeError(f"axon_start_nrt_profile rc={rc}")
                    try:
                        yield
                    finally:
                        lib.axon_stop_nrt_profile(str(output_dir).encode())

                state["hook"] = _hook
        except OSError:
            pass
        mod.get_axon_ntff_profile_hook = lambda: state["hook"]
        mod.set_axon_ntff_profile_hook = lambda h: state.update(hook=h)
        sys.modules["antenv.axon_hooks"] = mod
        try:
            import antenv

            antenv.axon_hooks = mod
        except ImportError:
            pass

    import concourse.bass_utils as bu

    if not getattr(bu.upload_artifacts, "_safe_wrapped", False):
        orig = bu.upload_artifacts

        def _safe_upload(tmpdir):
            try:
                return orig(tmpdir)
            except Exception:
                return f"local://{tmpdir}"

        _safe_upload._safe_wrapped = True
        bu.upload_artifacts = _safe_upload


def kernel(**inputs: np.ndarray) -> np.ndarray:
    global LAST_RESULTS
    _harden_trace_path()
    from concourse.bass_utils import run_bass_kernel_spmd

    mode = WEIGHT_MODE
    x = np.asarray(inputs["x"], dtype=np.float32)
    expert_indices = np.asarray(inputs["expert_indices"]).astype(np.int64)
    w1 = np.asarray(inputs["w1"], dtype=np.float32)
    w2 = np.asarray(inputs["w2"], dtype=np.float32)
    w3 = np.asarray(inputs["w3"], dtype=np.float32)

    np_wdt = _np_wdt(mode)

    # x.T laid out [128 partitions, 8 k-chunks * 16 tokens]
    xt = np.ascontiguousarray(
        x.T.reshape(KD, P, T).transpose(1, 0, 2), dtype=np_wdt
    ).reshape(P, KD * T)
    eye = np.eye(T, dtype=np.float32)

    in_maps = []
    for e in range(N_CORES):
        in_maps.append(
            {
                "xt": xt,
                "w13p": _pack_w13(w1[e], w3[e], np_wdt),
                "w2p": _pack_w2(w2[e], np_wdt),
                "eye": eye,
            }
        )

    nc = _get_nc(mode)
    res = run_bass_kernel_spmd(nc, in_maps, core_ids=list(range(N_CORES)))
    LAST_RESULTS = res

    y = np.stack([res.results[e]["out"] for e in range(N_CORES)])  # [8, 16, 1024]
    out = y[expert_indices, np.arange(T)[:, None]]  # [16, 2, 1024]
    return np.ascontiguousarray(out, dtype=np.float32)
